# revision 1
# baseline (speedup 1.0000x reference)
"""Trainium2 Bass kernel for nn_Attention_78314433675979 (sparse windowed attention).

Contract: kernel(**inputs) takes the FULL unsharded inputs (same keys as
reference.setup_inputs()) and returns the full outputs (tuple matching
reference()). Internally shards batch dim across 8 NeuronCores (data
parallel), runs a Bass/Tile kernel via run_bass_kernel_spmd, and
re-assembles on host.

Device computation per core (BL=16 local batches, H=128 on partitions):
  q       = Wq @ query + (bq + conv_b)              (PE, fp32)
  loc     = conv1d(cum_window)                      (PE, f32r, im2col from host)
  feat    = tanh(loc + q)                           (ACT, fused bias)
  score   = Ws . feat  -> [16b, 129l] via block-diagonal accumulation (PE)
  align   = softmax(score + mask)                   (DVE/ACT, b on partitions)
  context = sum_l align * tokens_window             (PE, bf16, block-diag lhsT)
  new_cum_window = cum_window + align               (DVE)
Host does: window gather/scatter (sharding glue), argmax/window-advance
index arithmetic, and batch shard/unshard.
"""

import os
import sys

import numpy as np

for _p in ("/opt/trn_rl_repo", "/root/.axon_site/_ro/trn_rl_repo"):
    if os.path.isdir(_p) and _p not in sys.path:
        sys.path.insert(0, _p)

import ml_dtypes  # noqa: E402
from contextlib import ExitStack  # noqa: E402

import concourse.bass as bass  # noqa: E402
import concourse.tile as tile  # noqa: E402
from concourse import bacc, mybir  # noqa: E402
from concourse.bass_utils import run_bass_kernel_spmd  # noqa: E402

# Problem constants (hardcoded per spec)
T, B, C = 600, 128, 512
QH, H = 1024, 128
K = 9
PAD = (K - 1) // 2  # 4
WL = 129
AVG = 1.5
N_CORES = 8
BL = B // N_CORES  # 16 batches per core

F32 = mybir.dt.float32
F32R = mybir.dt.float32r
BF16 = mybir.dt.bfloat16
I32 = mybir.dt.int32


def _emit_body(nc, tc, ctx, pools, aps, consts):
    """Emit one iteration of the device computation (everything that depends
    on per-call input data)."""
    work, big, psA, psConv, psS, psT, psC = pools
    (sb_wqt, sb_qt_unused, sb_cwt, sb_wsbig, sb_bvec, sb_id16) = consts
    AF = mybir.ActivationFunctionType

    # ---- per-iteration input loads ----
    sb_qt = work.tile([128, 128], F32, name="sb_qt")
    nc.sync.dma_start(out=sb_qt, in_=aps["qt"])
    sb_imc = work.tile([9, 2064], F32R, name="sb_imc")
    nc.sync.dma_start(out=sb_imc, in_=aps["imc"])
    sb_madd = work.tile([BL, WL], F32, name="sb_madd")
    nc.sync.dma_start(out=sb_madd, in_=aps["madd"])
    sb_cmid = work.tile([BL, WL], F32, name="sb_cmid")
    nc.sync.dma_start(out=sb_cmid, in_=aps["cmid"])

    # token windows: [16b, 129l, 512c] DRAM -> SBUF [128l, 16b, 512c] (l<128)
    sb_tok = big.tile([128, BL, C], BF16, name="sb_tok")
    for g in range(4):
        src = aps["tok"][g * 4:(g + 1) * 4, 0:128, :].rearrange("b l c -> l b c")
        nc.sync.dma_start(out=sb_tok[:, g * 4:(g + 1) * 4, :], in_=src)
    sb_tokl = work.tile([BL, C], BF16, name="sb_tokl")
    nc.sync.dma_start(out=sb_tokl, in_=aps["tok"][:, 128, :])

    # ---- q projection: psum_q[h, b] = sum_qh Wq[h,qh] query[b,qh] ----
    ps_q = psA.tile([128, BL], F32, name="ps_q")
    for j in range(8):
        nc.tensor.matmul(ps_q, lhsT=sb_wqt[:, j * 128:(j + 1) * 128],
                         rhs=sb_qt[:, j * BL:(j + 1) * BL],
                         start=(j == 0), stop=(j == 7))
    sb_bias = work.tile([128, BL], F32, name="sb_bias")
    nc.vector.tensor_scalar_add(sb_bias, ps_q, sb_bvec)

    # ---- conv (f32r, batch pairs, N=258) + fused bias+tanh per batch ----
    feat = big.tile([128, BL * WL], F32, name="feat")
    for p in range(8):
        ps_l = psConv.tile([128, 2 * WL], F32, name="ps_l")
        nc.tensor.matmul(ps_l, lhsT=sb_cwt,
                         rhs=sb_imc[:, p * 2 * WL:(p + 1) * 2 * WL],
                         start=True, stop=True)
        for hh in range(2):
            b = 2 * p + hh
            nc.scalar.activation(out=feat[:, b * WL:(b + 1) * WL],
                                 in_=ps_l[:, hh * WL:(hh + 1) * WL],
                                 func=AF.Tanh, bias=sb_bias[:, b:b + 1],
                                 scale=1.0)

    # ---- score[b, l] via block-diagonal accumulation over batches ----
    ps_sc = psS.tile([BL, WL], F32, name="ps_sc")
    for b in range(BL):
        nc.tensor.matmul(ps_sc, lhsT=sb_wsbig[:, 15 - b:31 - b],
                         rhs=feat[:, b * WL:(b + 1) * WL],
                         start=(b == 0), stop=(b == BL - 1))

    # ---- masked softmax on [16, 129] ----
    sb_scm = work.tile([BL, WL], F32, name="sb_scm")
    nc.vector.tensor_add(sb_scm, ps_sc, sb_madd)
    sb_negmax = work.tile([BL, 1], F32, name="sb_negmax")
    nc.vector.reduce_max(out=sb_negmax, in_=sb_scm,
                         axis=mybir.AxisListType.X, negate=True)
    sb_exp = work.tile([BL, WL], F32, name="sb_exp")
    sb_sum = work.tile([BL, 1], F32, name="sb_sum")
    nc.scalar.activation(out=sb_exp, in_=sb_scm, func=AF.Exp,
                         bias=sb_negmax, scale=1.0, accum_out=sb_sum)
    sb_rcp = work.tile([BL, 1], F32, name="sb_rcp")
    nc.vector.reciprocal(sb_rcp, sb_sum)
    sb_aln = work.tile([BL, WL], F32, name="sb_aln")
    nc.vector.tensor_scalar_mul(sb_aln, sb_exp, sb_rcp)
    nc.sync.dma_start(out=aps["aln_o"], in_=sb_aln)

    # new cum window
    sb_ncw = work.tile([BL, WL], F32, name="sb_ncw")
    nc.vector.tensor_add(sb_ncw, sb_aln, sb_cmid)
    nc.sync.dma_start(out=aps["ncw_o"], in_=sb_ncw)

    # ---- transpose align[:, :128] -> [128, 16]; build block-diag lhsT ----
    ps_t = psT.tile([128, BL], F32, name="ps_t")
    nc.tensor.transpose(ps_t, sb_aln[:, 0:128], sb_id16)
    sb_alnT = work.tile([128, BL], F32, name="sb_alnT")
    nc.vector.tensor_copy(out=sb_alnT, in_=ps_t)
    sb_atb = work.tile([128, 256], BF16, name="sb_atb")
    nc.vector.memset(sb_atb, 0.0)
    atb_cols = bass.AP(tensor=sb_atb.tensor, offset=sb_atb.offset,
                       ap=[sb_atb.ap[0], [17, 16]])
    nc.vector.tensor_copy(out=atb_cols, in_=sb_alnT)
    sb_diag = work.tile([BL, BL], BF16, name="sb_diag")
    nc.vector.tensor_scalar_mul(sb_diag, sb_id16, sb_aln[:, 128:129])

    # ---- context: 16 block-diag matmuls (l=0..127) + diag matmul (l=128) ----
    ps_ctx = psC.tile([BL, C], F32, name="ps_ctx")
    for g in range(BL):
        nc.tensor.matmul(ps_ctx, lhsT=sb_atb[:, 16 * g:16 * g + 16],
                         rhs=sb_tok[:, g, :], start=(g == 0), stop=False)
    nc.tensor.matmul(ps_ctx, lhsT=sb_diag, rhs=sb_tokl, start=False, stop=True)
    sb_ctx = work.tile([BL, C], F32, name="sb_ctx")
    nc.vector.tensor_copy(out=sb_ctx, in_=ps_ctx)
    nc.sync.dma_start(out=aps["ctx_o"], in_=sb_ctx)


def _build(loop_n=1):
    """Build + compile the SPMD NeuronCore program. loop_n > 1 wraps the body
    in a device-side For_i loop (used only for timing measurements)."""
    nc = bacc.Bacc("TRN2", target_bir_lowering=False, debug=False,
                   enable_asserts=True, num_devices=N_CORES)

    aps = {}
    aps["tok"] = nc.dram_tensor("tok", [BL, WL, C], BF16, kind="ExternalInput").ap()
    aps["imc"] = nc.dram_tensor("imc", [9, BL * WL], F32R, kind="ExternalInput").ap()
    aps["qt"] = nc.dram_tensor("qt", [128, 128], F32, kind="ExternalInput").ap()
    aps["wqt"] = nc.dram_tensor("wqt", [128, 1024], F32, kind="ExternalInput").ap()
    aps["cwt"] = nc.dram_tensor("cwt", [9, 128], F32R, kind="ExternalInput").ap()
    aps["wsv"] = nc.dram_tensor("wsv", [128, 1], F32, kind="ExternalInput").ap()
    aps["bvec"] = nc.dram_tensor("bvec", [128, 1], F32, kind="ExternalInput").ap()
    aps["madd"] = nc.dram_tensor("madd", [BL, WL], F32, kind="ExternalInput").ap()
    aps["cmid"] = nc.dram_tensor("cmid", [BL, WL], F32, kind="ExternalInput").ap()
    aps["ctx_o"] = nc.dram_tensor("ctx_o", [BL, C], F32, kind="ExternalOutput").ap()
    aps["aln_o"] = nc.dram_tensor("aln_o", [BL, WL], F32, kind="ExternalOutput").ap()
    aps["ncw_o"] = nc.dram_tensor("ncw_o", [BL, WL], F32, kind="ExternalOutput").ap()

    with tile.TileContext(nc) as tc:
        with ExitStack() as ctx:
            const = ctx.enter_context(tc.tile_pool(name="const", bufs=1))
            work = ctx.enter_context(tc.tile_pool(name="work", bufs=1))
            big = ctx.enter_context(tc.tile_pool(name="big", bufs=1))
            psA = ctx.enter_context(tc.tile_pool(name="psA", bufs=1, space="PSUM"))
            psConv = ctx.enter_context(tc.tile_pool(name="psConv", bufs=2, space="PSUM"))
            psS = ctx.enter_context(tc.tile_pool(name="psS", bufs=1, space="PSUM"))
            psT = ctx.enter_context(tc.tile_pool(name="psT", bufs=1, space="PSUM"))
            psC = ctx.enter_context(tc.tile_pool(name="psC", bufs=1, space="PSUM"))
            pools = (work, big, psA, psConv, psS, psT, psC)

            # ---- constants (loaded once) ----
            sb_wqt = const.tile([128, 1024], F32, name="sb_wqt")
            nc.sync.dma_start(out=sb_wqt, in_=aps["wqt"])
            sb_cwt = const.tile([9, 128], F32R, name="sb_cwt")
            nc.sync.dma_start(out=sb_cwt, in_=aps["cwt"])
            sb_wsv = const.tile([128, 1], F32, name="sb_wsv")
            nc.sync.dma_start(out=sb_wsv, in_=aps["wsv"])
            sb_bvec = const.tile([128, 1], F32, name="sb_bvec")
            nc.sync.dma_start(out=sb_bvec, in_=aps["bvec"])
            # sliding-window Ws matrix: zeros [128, 31] with Ws at col 15
            sb_wsbig = const.tile([128, 31], F32, name="sb_wsbig")
            nc.vector.memset(sb_wsbig, 0.0)
            nc.vector.tensor_copy(out=sb_wsbig[:, 15:16], in_=sb_wsv)
            # identity [16, 16] via iota + is_equal
            r16 = const.tile([16, 16], I32, name="r16")
            c16 = const.tile([16, 16], I32, name="c16")
            nc.gpsimd.iota(r16, pattern=[[0, 16]], base=0, channel_multiplier=1)
            nc.gpsimd.iota(c16, pattern=[[1, 16]], base=0, channel_multiplier=0)
            sb_id16 = const.tile([16, 16], F32, name="sb_id16")
            nc.vector.tensor_tensor(sb_id16, r16, c16, mybir.AluOpType.is_equal)

            consts = (sb_wqt, None, sb_cwt, sb_wsbig, sb_bvec, sb_id16)

            if loop_n == 1:
                _emit_body(nc, tc, ctx, pools, aps, consts)
            else:
                with tc.For_i(0, loop_n, 1):
                    _emit_body(nc, tc, ctx, pools, aps, consts)

    nc.compile()
    return nc


_NC_CACHE = {}


def _get_nc(loop_n=1):
    if loop_n not in _NC_CACHE:
        _NC_CACHE[loop_n] = _build(loop_n)
    return _NC_CACHE[loop_n]


def _to_bf16(x):
    """Fast float32 -> bfloat16 round-to-nearest-even."""
    v = np.ascontiguousarray(x, np.float32).view(np.uint32)
    r = ((v + 0x7FFF + ((v >> 16) & 1)) >> 16).astype(np.uint16)
    return r.view(ml_dtypes.bfloat16)


def _prepare(inputs):
    tokens = np.asarray(inputs["tokens"], np.float32)          # [T, B, C]
    tokens_mask = np.asarray(inputs["tokens_mask"]).astype(bool)  # [B, T]
    query = np.asarray(inputs["query"], np.float32)            # [1, B, QH]
    cum = np.asarray(inputs["cum_alignment"], np.float32)      # [B, T+2P]
    ws = np.asarray(inputs["window_start"]).astype(np.int64)   # [B]
    conv_w = np.asarray(inputs["conv_w"], np.float32)          # [H, 1, K]
    conv_b = np.asarray(inputs["conv_b"], np.float32)          # [H]
    Wq = np.asarray(inputs["Wq"], np.float32)                  # [H, QH]
    bq = np.asarray(inputs["bq"], np.float32)                  # [H]
    Ws = np.asarray(inputs["Ws"], np.float32)                  # [H]

    idx = ws[:, None] + np.arange(WL)                          # [B, wl]
    idx_cl = np.clip(idx, 0, T - 1)                            # jax clamps gathers
    mask_w = np.take_along_axis(tokens_mask, idx_cl, axis=1)   # [B, wl]
    tokens_w = tokens[idx_cl, np.arange(B)[:, None], :]        # [B, wl, C]
    tokens_w_bf = _to_bf16(tokens_w)
    idx_c = ws[:, None] + np.arange(WL + 2 * PAD)
    idx_ccl = np.clip(idx_c, 0, T + 2 * PAD - 1)
    cum_w = np.take_along_axis(cum, idx_ccl, axis=1)           # [B, wl+2P]
    loc_in = cum_w / np.float32(AVG) - np.float32(1.0)         # [B, wl+2P]
    # im2col [9, B, wl]
    imc = np.stack([loc_in[:, k:k + WL] for k in range(K)], axis=0)
    madd = np.where(mask_w, np.float32(0.0), np.float32(-1e30)).astype(np.float32)
    cmid = np.ascontiguousarray(cum_w[:, PAD:PAD + WL])        # [B, wl]

    # qt per core: qt[p, 16j + b] = query[0, b_global, 128j + p]
    # wqt (shared): wqt[p, 128j + h] = Wq[h, 128j + p]
    wqt = np.ascontiguousarray(
        Wq.T.reshape(8, 128, H).transpose(1, 0, 2).reshape(128, 8 * H))
    cwt = np.ascontiguousarray(conv_w[:, 0, :].T)              # [9, H]
    wsv = np.ascontiguousarray(Ws.reshape(H, 1))
    bvec = np.ascontiguousarray((bq + conv_b).reshape(H, 1))

    in_maps = []
    for core in range(N_CORES):
        bs = slice(core * BL, (core + 1) * BL)
        q_core = query[0, bs, :]                               # [BL, QH]
        qt = np.ascontiguousarray(
            q_core.T.reshape(8, 128, BL).transpose(1, 0, 2).reshape(128, 8 * BL))
        in_maps.append({
            "tok": np.ascontiguousarray(tokens_w_bf[bs]),
            "imc": np.ascontiguousarray(
                imc[:, bs, :].reshape(K, BL * WL)),
            "qt": qt,
            "wqt": wqt,
            "cwt": cwt,
            "wsv": wsv,
            "bvec": bvec,
            "madd": np.ascontiguousarray(madd[bs]),
            "cmid": np.ascontiguousarray(cmid[bs]),
        })
    host_ctx = {
        "idx": idx, "cum": cum, "ws_dtype": np.asarray(inputs["window_start"]).dtype,
        "window_start": np.asarray(inputs["window_start"]),
        "num_tokens": np.asarray(inputs["num_tokens"]),
    }
    return in_maps, host_ctx


def _assemble(results, host_ctx):
    ctx_out = np.concatenate([r["ctx_o"] for r in results], axis=0)   # [B, C]
    aln = np.concatenate([r["aln_o"] for r in results], axis=0)       # [B, wl]
    ncw = np.concatenate([r["ncw_o"] for r in results], axis=0)       # [B, wl]

    idx = host_ctx["idx"]
    cum = host_ctx["cum"]
    full_len = T + 2 * PAD
    pos = idx + PAD                                                    # [B, wl]
    full = np.zeros((B, full_len), np.float32)
    new_cum = cum.copy()
    inb = pos < full_len  # jax scatter drops OOB
    if inb.all():
        bidx = np.arange(B)[:, None]
        full[bidx, pos] = aln
        new_cum[bidx, pos] = ncw
    else:
        for b in range(B):
            p, v = pos[b][inb[b]], aln[b][inb[b]]
            full[b, p] = v
            new_cum[b, p] = ncw[b][inb[b]]

    out_aln = full[:, PAD:-PAD]                                        # [B, T]
    ws_in = host_ctx["window_start"]
    num_tokens = host_ctx["num_tokens"]
    amax = np.argmax(full, axis=1)
    new_ws = amax.astype(np.int64) - WL // 2 - PAD
    new_ws = np.minimum(new_ws, np.asarray(num_tokens, np.int64) - WL)
    new_ws = np.clip(new_ws, 0, None)
    new_ws = np.maximum(np.asarray(ws_in, np.int64), new_ws)
    new_ws = new_ws.astype(host_ctx["ws_dtype"])
    return ctx_out, out_aln, new_cum, new_ws


def kernel(**inputs):
    nc = _get_nc(loop_n=1)
    in_maps, host_ctx = _prepare(inputs)
    res = run_bass_kernel_spmd(nc, in_maps, core_ids=list(range(N_CORES)))
    return _assemble(res.results, host_ctx)


# revision 5
# speedup vs baseline: 1.0427x; 1.0427x over previous
"""Trainium2 Bass kernel for nn_Attention_78314433675979 (sparse windowed attention).

Contract: kernel(**inputs) takes the FULL unsharded inputs (same keys as
reference.setup_inputs()) and returns the full outputs (tuple matching
reference()). Internally shards the batch dim across 8 NeuronCores (data
parallel), runs a Bass/Tile kernel via run_bass_kernel_spmd, and
re-assembles on host.

Device computation per core (BL=16 local batches, H=128 on partitions):
  qT[b,h]  = query @ Wq.T + (bq + conv_b)                (PE fp16, psum f32,
             bias folded in via a ones-row matmul)
  loc+bias = conv1d(cum_window) + qT-broadcast           (PE f32r: conv matmul
             + selector matmul that broadcasts q over the window dim)
  feat     = tanh(loc+bias)                              (ACT, pairs of batches)
  score    = Ws . feat -> [16b, 129l]                    (PE fp16, block-diag
             accumulation via sliding-window Ws matrix)
  align    = softmax(score + mask)                       (DVE/ACT, b on parts)
  context  = sum_l align * tokens_window                 (PE fp16, block-diag
             lhsT built by strided-column scatter + diag matmul for l=128)
  new_cum_window = cum_window + align                    (DVE)
Host does: window gather/scatter (sharding glue), argmax/window-advance index
arithmetic, and batch shard/unshard.
"""

import os
import sys

import numpy as np

for _p in ("/opt/trn_rl_repo", "/root/.axon_site/_ro/trn_rl_repo"):
    if os.path.isdir(_p) and _p not in sys.path:
        sys.path.insert(0, _p)

from contextlib import ExitStack  # noqa: E402

import concourse.bass as bass  # noqa: E402
import concourse.tile as tile  # noqa: E402
from concourse import bacc, mybir  # noqa: E402
from concourse.bass_utils import run_bass_kernel_spmd  # noqa: E402

# Problem constants (hardcoded per spec)
T, B, C = 600, 128, 512
QH, H = 1024, 128
K = 9
PAD = (K - 1) // 2  # 4
WL = 129
AVG = 1.5
N_CORES = 8
BL = B // N_CORES  # 16 batches per core

F32 = mybir.dt.float32
F32R = mybir.dt.float32r
F16 = mybir.dt.float16
I32 = mybir.dt.int32

# wqx layout: [128, 1024 wq | 128 bvecT(row0) | 1 Ws] fp16
WQX_COLS = 1024 + 128 + 1
# cblob layout: [16, 2064 sel | 128 cwt(rows 0..8)] fp16
CBLOB_COLS = BL * WL + 128


def _emit_body(nc, tc, pools, aps, consts):
    """One iteration of the device computation (everything data-dependent)."""
    work, big, psA, psConv, psS, psT, psC = pools
    sb_wqx, sb_cblob, sb_wsbig, sb_ones, sb_id16, sb_id128 = consts
    AF = mybir.ActivationFunctionType
    sb_sel = sb_cblob[:, 0:BL * WL]
    sb_cwt = sb_cblob[0:K, BL * WL:BL * WL + 128]

    # ---- per-iteration input loads (small criticals first) ----
    sb_qt = work.tile([128, 8 * BL], F16, name="sb_qt")
    d_qt = nc.sync.dma_start(out=sb_qt, in_=aps["qt"])
    sb_imc = work.tile([K, BL * WL], F16, name="sb_imc")
    d_imc = nc.sync.dma_start(out=sb_imc, in_=aps["imc"])
    # madd | cmid packed [16, 258]
    sb_mdc = work.tile([BL, 2 * WL], F32, name="sb_mdc")
    d_mdc = nc.sync.dma_start(out=sb_mdc, in_=aps["mdc"])
    sb_tokl = work.tile([BL, C], F16, name="sb_tokl")
    nc.scalar.dma_start(out=sb_tokl, in_=aps["tok"][128, :, :])
    # big token-window load: [128l, 16b, 512c] fp16, one SWDGE DMA on the
    # otherwise-idle Pool engine (contiguous 16 KiB per partition row).
    # Explicitly ordered after the small critical input DMAs so its ~6us
    # transfer doesn't starve them on the shared SDMA engines.
    sb_tok = big.tile([128, BL, C], F16, name="sb_tok")
    d_tok = nc.gpsimd.dma_start(out=sb_tok, in_=aps["tok"][0:128, :, :])
    for d in (d_qt, d_imc, d_mdc):
        tile.add_dep_helper(d.ins, d_tok.ins, sync=True,
                            reason="small input DMAs before bulk token DMA")

    # ---- qT[b, h] = query @ Wq.T + (bq + conv_b), directly in [16, 128] ----
    ps_qT = psA.tile([BL, 128], F32, name="ps_qT")
    for j in range(8):
        nc.tensor.matmul(ps_qT, lhsT=sb_qt[:, j * BL:(j + 1) * BL],
                         rhs=sb_wqx[:, j * 128:(j + 1) * 128],
                         start=(j == 0), stop=False)
    nc.tensor.matmul(ps_qT, lhsT=sb_ones, rhs=sb_wqx[0:1, 1024:1152],
                     start=False, stop=True)
    sb_qTr = work.tile([BL, 128], F16, name="sb_qTr")
    nc.vector.tensor_copy(out=sb_qTr, in_=ps_qT)

    # ---- conv (f32r, batch pairs, N=258) + bias-inject + tanh ----
    feat = big.tile([128, BL * WL], F16, name="feat")
    for p in range(8):
        ps_l = psConv.tile([128, 2 * WL], F32, name="ps_l")
        nc.tensor.matmul(ps_l, lhsT=sb_cwt,
                         rhs=sb_imc[:, p * 2 * WL:(p + 1) * 2 * WL],
                         start=True, stop=False)
        nc.tensor.matmul(ps_l, lhsT=sb_qTr,
                         rhs=sb_sel[:, p * 2 * WL:(p + 1) * 2 * WL],
                         start=False, stop=True)
        nc.scalar.activation(out=feat[:, p * 2 * WL:(p + 1) * 2 * WL],
                             in_=ps_l, func=AF.Tanh)

    # ---- score[b, l] via block-diagonal accumulation over batches ----
    ps_sc = psS.tile([BL, WL], F32, name="ps_sc")
    for b in range(BL):
        nc.tensor.matmul(ps_sc, lhsT=sb_wsbig[:, 15 - b:31 - b],
                         rhs=feat[:, b * WL:(b + 1) * WL],
                         start=(b == 0), stop=(b == BL - 1))

    # ---- masked softmax on [16, 129] ----
    sb_scm = work.tile([BL, WL], F32, name="sb_scm")
    nc.vector.tensor_add(sb_scm, ps_sc, sb_mdc[:, 0:WL])
    sb_negmax = work.tile([BL, 1], F32, name="sb_negmax")
    nc.vector.reduce_max(out=sb_negmax, in_=sb_scm,
                         axis=mybir.AxisListType.X, negate=True)
    sb_exp = work.tile([BL, WL], F32, name="sb_exp")
    sb_sum = work.tile([BL, 1], F32, name="sb_sum")
    nc.scalar.activation(out=sb_exp, in_=sb_scm, func=AF.Exp,
                         bias=sb_negmax, scale=1.0, accum_out=sb_sum)
    sb_rcp = work.tile([BL, 1], F32, name="sb_rcp")
    nc.vector.reciprocal(sb_rcp, sb_sum)
    # align | new-cum-window packed [16, 258] -> one output DMA
    sb_anw = work.tile([BL, 2 * WL], F32, name="sb_anw")
    aln = sb_anw[:, 0:WL]
    nc.vector.tensor_scalar_mul(aln, sb_exp, sb_rcp)
    nc.vector.tensor_add(sb_anw[:, WL:2 * WL], aln, sb_mdc[:, WL:2 * WL])
    nc.scalar.dma_start(out=aps["anw_o"], in_=sb_anw)

    # ---- transpose align[:, :128] -> [128, 16]; build block-diag lhsT ----
    ps_t = psT.tile([128, BL], F32, name="ps_t")
    nc.tensor.transpose(ps_t, aln[:, 0:128], sb_id16)
    sb_alnT = work.tile([128, BL], F32, name="sb_alnT")
    nc.vector.tensor_copy(out=sb_alnT, in_=ps_t)
    sb_atb = work.tile([128, 256], F16, name="sb_atb")
    nc.vector.memset(sb_atb, 0.0)
    atb_cols = bass.AP(tensor=sb_atb.tensor, offset=sb_atb.offset,
                       ap=[sb_atb.ap[0], [17, 16]])
    nc.vector.tensor_copy(out=atb_cols, in_=sb_alnT)
    sb_diag = work.tile([BL, BL], F16, name="sb_diag")
    nc.vector.tensor_scalar_mul(sb_diag, sb_id16, aln[:, 128:129])

    # ---- context: 16 block-diag matmuls (l=0..127) + diag matmul (l=128) ----
    ps_ctx = psC.tile([BL, C], F32, name="ps_ctx")
    for g in range(BL):
        nc.tensor.matmul(ps_ctx, lhsT=sb_atb[:, 16 * g:16 * g + 16],
                         rhs=sb_tok[:, g, :], start=(g == 0), stop=False)
    nc.tensor.matmul(ps_ctx, lhsT=sb_diag, rhs=sb_tokl, start=False, stop=True)
    sb_ctx = work.tile([BL, C], F32, name="sb_ctx")
    nc.vector.tensor_copy(out=sb_ctx, in_=ps_ctx)
    nc.scalar.dma_start(out=aps["ctx_o"], in_=sb_ctx)


def _build(loop_n=1):
    """Build + compile the SPMD NeuronCore program. loop_n > 1 wraps the body
    in a device-side For_i loop (used only for timing measurements)."""
    nc = bacc.Bacc("TRN2", target_bir_lowering=False, debug=False,
                   enable_asserts=True, num_devices=N_CORES)

    aps = {}
    aps["tok"] = nc.dram_tensor("tok", [WL, BL, C], F16, kind="ExternalInput").ap()
    aps["imc"] = nc.dram_tensor("imc", [K, BL * WL], F16, kind="ExternalInput").ap()
    aps["qt"] = nc.dram_tensor("qt", [128, 8 * BL], F16, kind="ExternalInput").ap()
    aps["wqx"] = nc.dram_tensor("wqx", [128, WQX_COLS], F16, kind="ExternalInput").ap()
    aps["cblob"] = nc.dram_tensor("cblob", [BL, CBLOB_COLS], F16, kind="ExternalInput").ap()
    aps["mdc"] = nc.dram_tensor("mdc", [BL, 2 * WL], F32, kind="ExternalInput").ap()
    aps["ctx_o"] = nc.dram_tensor("ctx_o", [BL, C], F32, kind="ExternalOutput").ap()
    aps["anw_o"] = nc.dram_tensor("anw_o", [BL, 2 * WL], F32, kind="ExternalOutput").ap()

    with tile.TileContext(nc) as tc:
        with ExitStack() as ctx:
            const = ctx.enter_context(tc.tile_pool(name="const", bufs=1))
            work = ctx.enter_context(tc.tile_pool(name="work", bufs=2))
            big = ctx.enter_context(tc.tile_pool(name="big", bufs=2))
            psA = ctx.enter_context(tc.tile_pool(name="psA", bufs=1, space="PSUM"))
            psConv = ctx.enter_context(tc.tile_pool(name="psConv", bufs=3, space="PSUM"))
            psS = ctx.enter_context(tc.tile_pool(name="psS", bufs=1, space="PSUM"))
            psT = ctx.enter_context(tc.tile_pool(name="psT", bufs=1, space="PSUM"))
            psC = ctx.enter_context(tc.tile_pool(name="psC", bufs=1, space="PSUM"))
            pools = (work, big, psA, psConv, psS, psT, psC)

            # warm the ACT function table (exp_and_others covers Tanh+Exp)
            # before any real dependency chain exists.
            scrap = const.tile([1, 2], F32, name="scrap")
            nc.vector.memset(scrap, 0.0)
            scrap2 = const.tile([1, 2], F32, name="scrap2")
            nc.scalar.activation(out=scrap2, in_=scrap,
                                 func=mybir.ActivationFunctionType.Exp)

            # ---- constants (loaded once; on the ACT HWDGE ring) ----
            sb_wqx = const.tile([128, WQX_COLS], F16, name="sb_wqx")
            nc.scalar.dma_start(out=sb_wqx, in_=aps["wqx"])
            sb_cblob = const.tile([BL, CBLOB_COLS], F16, name="sb_cblob")
            nc.scalar.dma_start(out=sb_cblob, in_=aps["cblob"])
            # sliding-window Ws matrix: zeros [128, 31] with Ws at col 15 (fp16)
            sb_wsbig = const.tile([128, 31], F16, name="sb_wsbig")
            nc.vector.memset(sb_wsbig, 0.0)
            nc.vector.tensor_copy(out=sb_wsbig[:, 15:16],
                                  in_=sb_wqx[:, 1152:1153])
            sb_ones = const.tile([1, BL], F16, name="sb_ones")
            nc.vector.memset(sb_ones, 1.0)
            # identities via iota + is_equal
            r16 = const.tile([16, 16], I32, name="r16")
            c16 = const.tile([16, 16], I32, name="c16")
            nc.gpsimd.iota(r16, pattern=[[0, 16]], base=0, channel_multiplier=1)
            nc.gpsimd.iota(c16, pattern=[[1, 16]], base=0, channel_multiplier=0)
            sb_id16 = const.tile([16, 16], F32, name="sb_id16")
            nc.vector.tensor_tensor(sb_id16, r16, c16, mybir.AluOpType.is_equal)
            r128 = const.tile([128, 128], I32, name="r128")
            c128 = const.tile([128, 128], I32, name="c128")
            nc.gpsimd.iota(r128, pattern=[[0, 128]], base=0, channel_multiplier=1)
            nc.gpsimd.iota(c128, pattern=[[1, 128]], base=0, channel_multiplier=0)
            sb_id128 = const.tile([128, 128], F32, name="sb_id128")
            nc.vector.tensor_tensor(sb_id128, r128, c128, mybir.AluOpType.is_equal)

            consts = (sb_wqx, sb_cblob, sb_wsbig, sb_ones, sb_id16, sb_id128)

            if loop_n == 1:
                _emit_body(nc, tc, pools, aps, consts)
            else:
                with tc.For_i(0, loop_n, 1):
                    _emit_body(nc, tc, pools, aps, consts)

    nc.compile()
    return nc


_NC_CACHE = {}


def _get_nc(loop_n=1):
    if loop_n not in _NC_CACHE:
        _NC_CACHE[loop_n] = _build(loop_n)
    return _NC_CACHE[loop_n]


def _prepare(inputs):
    tokens = np.asarray(inputs["tokens"], np.float32)          # [T, B, C]
    tokens_mask = np.asarray(inputs["tokens_mask"]).astype(bool)  # [B, T]
    query = np.asarray(inputs["query"], np.float32)            # [1, B, QH]
    cum = np.asarray(inputs["cum_alignment"], np.float32)      # [B, T+2P]
    ws = np.asarray(inputs["window_start"]).astype(np.int64)   # [B]
    conv_w = np.asarray(inputs["conv_w"], np.float32)          # [H, 1, K]
    conv_b = np.asarray(inputs["conv_b"], np.float32)          # [H]
    Wq = np.asarray(inputs["Wq"], np.float32)                  # [H, QH]
    bq = np.asarray(inputs["bq"], np.float32)                  # [H]
    Ws = np.asarray(inputs["Ws"], np.float32)                  # [H]

    idx = ws[:, None] + np.arange(WL)                          # [B, wl]
    idx_cl = np.clip(idx, 0, T - 1)                            # jax clamps gathers
    mask_w = np.take_along_axis(tokens_mask, idx_cl, axis=1)   # [B, wl]
    # l-major gathered token windows, fp16: [wl, B, C]
    tokens_w = tokens[idx_cl.T, np.arange(B)[None, :], :].astype(np.float16)
    idx_c = ws[:, None] + np.arange(WL + 2 * PAD)
    idx_ccl = np.clip(idx_c, 0, T + 2 * PAD - 1)
    cum_w = np.take_along_axis(cum, idx_ccl, axis=1)           # [B, wl+2P]
    loc_in = cum_w / np.float32(AVG) - np.float32(1.0)         # [B, wl+2P]
    # im2col [9, B, wl]
    imc = np.stack([loc_in[:, k:k + WL] for k in range(K)], axis=0).astype(np.float16)
    madd = np.where(mask_w, np.float32(0.0), np.float32(-1e30)).astype(np.float32)
    cmid = cum_w[:, PAD:PAD + WL]                               # [B, wl]
    mdc = np.concatenate([madd, cmid], axis=1)                  # [B, 258]

    # wqx (shared): [p, 128j + h] = Wq[h, 128j + p]; col 1024..1151 row 0 =
    # (bq + conv_b); col 1152 = Ws
    wqx = np.zeros((128, WQX_COLS), np.float16)
    wqx[:, 0:1024] = Wq.T.reshape(8, 128, H).transpose(1, 0, 2).reshape(128, 8 * H)
    wqx[0, 1024:1152] = (bq + conv_b).astype(np.float16)
    wqx[:, 1152] = Ws.astype(np.float16)
    # cblob (shared): [16, sel | cwt]
    cblob = np.zeros((BL, CBLOB_COLS), np.float16)
    for k in range(BL):
        cblob[k, k * WL:(k + 1) * WL] = 1.0
    cblob[0:K, BL * WL:BL * WL + 128] = conv_w[:, 0, :].T.astype(np.float16)

    in_maps = []
    for core in range(N_CORES):
        bs = slice(core * BL, (core + 1) * BL)
        q_core = query[0, bs, :]                               # [BL, QH]
        qt = np.ascontiguousarray(
            q_core.T.reshape(8, 128, BL).transpose(1, 0, 2).reshape(128, 8 * BL)
        ).astype(np.float16)
        in_maps.append({
            "tok": np.ascontiguousarray(tokens_w[:, bs, :]),
            "imc": np.ascontiguousarray(imc[:, bs, :].reshape(K, BL * WL)),
            "qt": qt,
            "wqx": wqx,
            "cblob": cblob,
            "mdc": np.ascontiguousarray(mdc[bs]),
        })
    host_ctx = {
        "idx": idx, "cum": cum, "ws_dtype": np.asarray(inputs["window_start"]).dtype,
        "window_start": np.asarray(inputs["window_start"]),
        "num_tokens": np.asarray(inputs["num_tokens"]),
    }
    return in_maps, host_ctx


def _assemble(results, host_ctx):
    ctx_out = np.concatenate([r["ctx_o"] for r in results], axis=0)   # [B, C]
    anw = np.concatenate([r["anw_o"] for r in results], axis=0)       # [B, 258]
    aln = anw[:, 0:WL]
    ncw = anw[:, WL:2 * WL]

    idx = host_ctx["idx"]
    cum = host_ctx["cum"]
    full_len = T + 2 * PAD
    pos = idx + PAD                                                    # [B, wl]
    full = np.zeros((B, full_len), np.float32)
    new_cum = cum.copy()
    inb = pos < full_len  # jax scatter drops OOB
    if inb.all():
        bidx = np.arange(B)[:, None]
        full[bidx, pos] = aln
        new_cum[bidx, pos] = ncw
    else:
        for b in range(B):
            p = pos[b][inb[b]]
            full[b, p] = aln[b][inb[b]]
            new_cum[b, p] = ncw[b][inb[b]]

    out_aln = full[:, PAD:-PAD]                                        # [B, T]
    ws_in = host_ctx["window_start"]
    num_tokens = host_ctx["num_tokens"]
    amax = np.argmax(full, axis=1)
    new_ws = amax.astype(np.int64) - WL // 2 - PAD
    new_ws = np.minimum(new_ws, np.asarray(num_tokens, np.int64) - WL)
    new_ws = np.clip(new_ws, 0, None)
    new_ws = np.maximum(np.asarray(ws_in, np.int64), new_ws)
    new_ws = new_ws.astype(host_ctx["ws_dtype"])
    return ctx_out, out_aln, new_cum, new_ws


def kernel(**inputs):
    nc = _get_nc(loop_n=1)
    in_maps, host_ctx = _prepare(inputs)
    res = run_bass_kernel_spmd(nc, in_maps, core_ids=list(range(N_CORES)))
    return _assemble(res.results, host_ctx)


# revision 9
# speedup vs baseline: 1.4470x; 1.3878x over previous
"""Trainium2 Bass kernel for nn_Attention_78314433675979 (sparse windowed attention).

Contract: kernel(**inputs) takes the FULL unsharded inputs (same keys as
reference.setup_inputs()) and returns the full outputs (tuple matching
reference()). Internally shards the batch dim across 8 NeuronCores (data
parallel), runs a Bass/Tile kernel via run_bass_kernel_spmd, and
re-assembles on host.

Device computation per core (BL=16 local batches, H=128 on partitions):
  qT[b,h]    = query @ Wq.T + (bq + conv_b)      (PE fp16 -> psum rows 32..47;
               bias folded in via a ones-row matmul)
  loc+bias   = one augmented matmul per 512-col chunk: lhsT cq[48,128] holds
               conv taps (rows 0..8) and qT (rows 32..47); rhs cimc[48,2064]
               holds im2col (rows 0..8) and a 0/1 batch selector (rows 32..47)
               that broadcasts q over the window dim.          (PE fp16)
  feat       = tanh(loc+bias)                    (ACT, 512-col chunks)
  score      = Ws . feat -> [16b, 129l]          (PE fp16, block-diagonal
               accumulation via sliding-window Ws matrix)
  e          = exp(score + mask - max), s = sum(e)  (DVE/ACT, b on partitions)
  context    = (sum_l e * tokens_window) / s     (PE fp16 via block-diag lhsT
               built by strided-column scatter; l=128 via diag matmul;
               normalization fused into the psum->sbuf copy)
  align      = e / s; new_cum_window = cum_window + align   (DVE, off critical
               path, packed into one output DMA)
Host does: window gather/scatter (sharding glue), argmax/window-advance index
arithmetic, and batch shard/unshard.
"""

import os
import sys

import numpy as np

for _p in ("/opt/trn_rl_repo", "/root/.axon_site/_ro/trn_rl_repo"):
    if os.path.isdir(_p) and _p not in sys.path:
        sys.path.insert(0, _p)

from contextlib import ExitStack  # noqa: E402

import concourse.bass as bass  # noqa: E402
import concourse.tile as tile  # noqa: E402
from concourse import bacc, mybir  # noqa: E402
from concourse.bass_utils import run_bass_kernel_spmd  # noqa: E402

# Problem constants (hardcoded per spec)
T, B, C = 600, 128, 512
QH, H = 1024, 128
K = 9
PAD = (K - 1) // 2  # 4
WL = 129
AVG = 1.5
N_CORES = 8
BL = B // N_CORES  # 16 batches per core

F32 = mybir.dt.float32
F16 = mybir.dt.float16
I32 = mybir.dt.int32

# wqx layout (all fp16): [128, 1024 Wq | 128 bvecT(row0) | 1 Ws | 128 cwt(rows 0..8)]
WQX_COLS = 1024 + 128 + 1 + 128
# cimc layout (fp16): [48, 2064]; rows 0..8 im2col, rows 32..47 batch selector
CIMC_ROWS = 48
NBL = BL * WL  # 2064
CONV_CHUNKS = [(0, 512), (512, 512), (1024, 512), (1536, 512), (2048, 16)]


def _emit_body(nc, tc, pools, aps, consts, phases=0xFF):
    """One iteration of the device computation (everything data-dependent).
    phases: bitmask for perf bisection — 1 dmas, 2 qproj, 4 conv, 8 score,
    16 softmax, 32 atb, 64 ctx, 128 outputs."""
    work, big, psA, psConv, psS, psT, psC = pools
    sb_wqx, sb_wsbig, sb_ones, sb_id16 = consts
    AF = mybir.ActivationFunctionType

    # ---- per-iteration input loads (small criticals first) ----
    sb_qt = work.tile([128, 8 * BL], F16, name="sb_qt")
    d_qt = nc.sync.dma_start(out=sb_qt, in_=aps["qt"])
    sb_cimc = work.tile([CIMC_ROWS, NBL], F16, name="sb_cimc")
    d_cimc = nc.sync.dma_start(out=sb_cimc, in_=aps["cimc"])
    # madd | cmid packed [16, 258]
    sb_mdc = work.tile([BL, 2 * WL], F32, name="sb_mdc")
    d_mdc = nc.sync.dma_start(out=sb_mdc, in_=aps["mdc"])
    sb_tokl = work.tile([BL, C], F16, name="sb_tokl")
    nc.scalar.dma_start(out=sb_tokl, in_=aps["tok"][128, :, :])
    # big token-window load: [128l, 16b, 512c] fp16, one SWDGE DMA on the
    # otherwise-idle Pool engine (contiguous 16 KiB per partition row).
    # Explicitly ordered after the small critical input DMAs so its ~6us
    # transfer doesn't starve them on the shared SDMA engines.
    sb_tok = big.tile([128, BL, C], F16, name="sb_tok")
    d_tok = nc.gpsimd.dma_start(out=sb_tok, in_=aps["tok"][0:128, :, :])
    for d in (d_qt, d_cimc, d_mdc):
        tile.add_dep_helper(d_tok.ins, d.ins, sync=True,
                            reason="small input DMAs before bulk token DMA")

    if not phases & 2:
        return
    # ---- qT: psum rows 32..47 get query @ Wq.T + (bq + conv_b) ----
    ps_qT = psA.tile([CIMC_ROWS, 128], F32, name="ps_qT")
    q_out = ps_qT[32:48, :]
    for j in range(8):
        nc.tensor.matmul(q_out, lhsT=sb_qt[:, j * BL:(j + 1) * BL],
                         rhs=sb_wqx[:, j * 128:(j + 1) * 128],
                         start=(j == 0), stop=False)
    nc.tensor.matmul(q_out, lhsT=sb_ones, rhs=sb_wqx[0:1, 1024:1152],
                     start=False, stop=True)
    # combined conv lhsT: rows 0..8 conv taps, rows 32..47 qT, rest zero
    sb_cq = work.tile([CIMC_ROWS, 128], F16, name="sb_cq")
    nc.vector.memset(sb_cq, 0.0)
    nc.vector.tensor_copy(out=sb_cq[0:K, :], in_=sb_wqx[0:K, 1153:1281])
    nc.vector.tensor_copy(out=sb_cq[32:48, :], in_=q_out)

    if not phases & 4:
        return
    # ---- conv + bias-inject in one augmented matmul per chunk, + tanh ----
    feat = big.tile([128, NBL], F16, name="feat")
    for c0, cn in CONV_CHUNKS:
        ps_l = psConv.tile([128, 512], F32, name="ps_l")
        nc.tensor.matmul(ps_l[:, 0:cn], lhsT=sb_cq,
                         rhs=sb_cimc[:, c0:c0 + cn], start=True, stop=True)
        nc.scalar.activation(out=feat[:, c0:c0 + cn], in_=ps_l[:, 0:cn],
                             func=AF.Tanh)

    if not phases & 8:
        return
    # ---- score[b, l] via block-diagonal accumulation over batches ----
    ps_sc = psS.tile([BL, WL], F32, name="ps_sc")
    for b in range(BL):
        nc.tensor.matmul(ps_sc, lhsT=sb_wsbig[:, 15 - b:31 - b],
                         rhs=feat[:, b * WL:(b + 1) * WL],
                         start=(b == 0), stop=(b == BL - 1))

    if not phases & 16:
        return
    # ---- masked, max-shifted exp on [16, 129] (normalization postponed) ----
    sb_scm = work.tile([BL, WL], F32, name="sb_scm")
    nc.vector.tensor_add(sb_scm, ps_sc, sb_mdc[:, 0:WL])
    sb_negmax = work.tile([BL, 1], F32, name="sb_negmax")
    nc.vector.reduce_max(out=sb_negmax, in_=sb_scm,
                         axis=mybir.AxisListType.X, negate=True)
    sb_exp = work.tile([BL, WL], F32, name="sb_exp")
    sb_sum = work.tile([BL, 1], F32, name="sb_sum")
    nc.scalar.activation(out=sb_exp, in_=sb_scm, func=AF.Exp,
                         bias=sb_negmax, scale=1.0, accum_out=sb_sum)
    sb_rcp = work.tile([BL, 1], F32, name="sb_rcp")
    nc.vector.reciprocal(sb_rcp, sb_sum)

    if not phases & 32:
        return
    # ---- transpose exp[:, :128] -> [128, 16]; build block-diag lhsT ----
    ps_t = psT.tile([128, BL], F32, name="ps_t")
    nc.tensor.transpose(ps_t, sb_exp[:, 0:128], sb_id16)
    sb_atb = work.tile([128, 256], F16, name="sb_atb")
    nc.vector.memset(sb_atb, 0.0)
    atb_cols = bass.AP(tensor=sb_atb.tensor, offset=sb_atb.offset,
                       ap=[sb_atb.ap[0], [17, 16]])
    nc.vector.tensor_copy(out=atb_cols, in_=ps_t)
    sb_diag = work.tile([BL, BL], F16, name="sb_diag")
    nc.vector.tensor_scalar_mul(sb_diag, sb_id16, sb_exp[:, 128:129])

    # off the context critical path: align output + new-cum window, one DMA
    sb_anw = work.tile([BL, 2 * WL], F32, name="sb_anw")
    aln = sb_anw[:, 0:WL]
    if phases & 128:
        nc.vector.tensor_scalar_mul(aln, sb_exp, sb_rcp)
        nc.vector.tensor_add(sb_anw[:, WL:2 * WL], aln, sb_mdc[:, WL:2 * WL])
        nc.scalar.dma_start(out=aps["anw_o"], in_=sb_anw)

    if not phases & 64:
        return
    # ---- context: 16 block-diag matmuls (l=0..127) + diag matmul (l=128),
    # normalized by 1/sum during the psum -> sbuf copy ----
    ps_ctx = psC.tile([BL, C], F32, name="ps_ctx")
    for g in range(BL):
        nc.tensor.matmul(ps_ctx, lhsT=sb_atb[:, 16 * g:16 * g + 16],
                         rhs=sb_tok[:, g, :], start=(g == 0), stop=False)
    nc.tensor.matmul(ps_ctx, lhsT=sb_diag, rhs=sb_tokl, start=False, stop=True)
    if phases & 128:
        sb_ctx = work.tile([BL, C], F32, name="sb_ctx")
        nc.vector.tensor_scalar_mul(sb_ctx, ps_ctx, sb_rcp)
        nc.scalar.dma_start(out=aps["ctx_o"], in_=sb_ctx)


def _build(loop_n=1, phases=0xFF):
    """Build + compile the SPMD NeuronCore program. loop_n > 1 wraps the body
    in a device-side For_i loop (used only for timing measurements)."""
    nc = bacc.Bacc("TRN2", target_bir_lowering=False, debug=False,
                   enable_asserts=True, num_devices=N_CORES)

    aps = {}
    aps["tok"] = nc.dram_tensor("tok", [WL, BL, C], F16, kind="ExternalInput").ap()
    aps["cimc"] = nc.dram_tensor("cimc", [CIMC_ROWS, NBL], F16, kind="ExternalInput").ap()
    aps["qt"] = nc.dram_tensor("qt", [128, 8 * BL], F16, kind="ExternalInput").ap()
    aps["wqx"] = nc.dram_tensor("wqx", [128, WQX_COLS], F16, kind="ExternalInput").ap()
    aps["mdc"] = nc.dram_tensor("mdc", [BL, 2 * WL], F32, kind="ExternalInput").ap()
    aps["ctx_o"] = nc.dram_tensor("ctx_o", [BL, C], F32, kind="ExternalOutput").ap()
    aps["anw_o"] = nc.dram_tensor("anw_o", [BL, 2 * WL], F32, kind="ExternalOutput").ap()

    with tile.TileContext(nc) as tc:
        with ExitStack() as ctx:
            const = ctx.enter_context(tc.tile_pool(name="const", bufs=1))
            work = ctx.enter_context(tc.tile_pool(name="work", bufs=2))
            big = ctx.enter_context(tc.tile_pool(name="big", bufs=2))
            psA = ctx.enter_context(tc.tile_pool(name="psA", bufs=1, space="PSUM"))
            psConv = ctx.enter_context(tc.tile_pool(name="psConv", bufs=3, space="PSUM"))
            psS = ctx.enter_context(tc.tile_pool(name="psS", bufs=1, space="PSUM"))
            psT = ctx.enter_context(tc.tile_pool(name="psT", bufs=1, space="PSUM"))
            psC = ctx.enter_context(tc.tile_pool(name="psC", bufs=1, space="PSUM"))
            pools = (work, big, psA, psConv, psS, psT, psC)

            # warm the ACT function table (exp_and_others covers Tanh+Exp)
            # before any real dependency chain exists.
            scrap = const.tile([1, 2], F32, name="scrap")
            nc.vector.memset(scrap, 0.0)
            scrap2 = const.tile([1, 2], F32, name="scrap2")
            nc.scalar.activation(out=scrap2, in_=scrap,
                                 func=mybir.ActivationFunctionType.Exp)

            # ---- constants (loaded once; on the ACT HWDGE ring) ----
            sb_wqx = const.tile([128, WQX_COLS], F16, name="sb_wqx")
            nc.scalar.dma_start(out=sb_wqx, in_=aps["wqx"])
            # sliding-window Ws matrix: zeros [128, 31] with Ws at col 15 (fp16)
            sb_wsbig = const.tile([128, 31], F16, name="sb_wsbig")
            nc.vector.memset(sb_wsbig, 0.0)
            nc.vector.tensor_copy(out=sb_wsbig[:, 15:16],
                                  in_=sb_wqx[:, 1152:1153])
            sb_ones = const.tile([1, BL], F16, name="sb_ones")
            nc.vector.memset(sb_ones, 1.0)
            # identity via iota + is_equal
            r16 = const.tile([16, 16], I32, name="r16")
            c16 = const.tile([16, 16], I32, name="c16")
            nc.gpsimd.iota(r16, pattern=[[0, 16]], base=0, channel_multiplier=1)
            nc.gpsimd.iota(c16, pattern=[[1, 16]], base=0, channel_multiplier=0)
            sb_id16 = const.tile([16, 16], F32, name="sb_id16")
            nc.vector.tensor_tensor(sb_id16, r16, c16, mybir.AluOpType.is_equal)

            consts = (sb_wqx, sb_wsbig, sb_ones, sb_id16)

            if loop_n == 1:
                _emit_body(nc, tc, pools, aps, consts, phases)
            else:
                with tc.For_i(0, loop_n, 1):
                    _emit_body(nc, tc, pools, aps, consts, phases)

    nc.compile()
    return nc


_NC_CACHE = {}


def _get_nc(loop_n=1, phases=0xFF):
    key = (loop_n, phases)
    if key not in _NC_CACHE:
        _NC_CACHE[key] = _build(loop_n, phases)
    return _NC_CACHE[key]


def _prepare(inputs):
    tokens = np.asarray(inputs["tokens"], np.float32)          # [T, B, C]
    tokens_mask = np.asarray(inputs["tokens_mask"]).astype(bool)  # [B, T]
    query = np.asarray(inputs["query"], np.float32)            # [1, B, QH]
    cum = np.asarray(inputs["cum_alignment"], np.float32)      # [B, T+2P]
    ws = np.asarray(inputs["window_start"]).astype(np.int64)   # [B]
    conv_w = np.asarray(inputs["conv_w"], np.float32)          # [H, 1, K]
    conv_b = np.asarray(inputs["conv_b"], np.float32)          # [H]
    Wq = np.asarray(inputs["Wq"], np.float32)                  # [H, QH]
    bq = np.asarray(inputs["bq"], np.float32)                  # [H]
    Ws = np.asarray(inputs["Ws"], np.float32)                  # [H]

    idx = ws[:, None] + np.arange(WL)                          # [B, wl]
    idx_cl = np.clip(idx, 0, T - 1)                            # jax clamps gathers
    mask_w = np.take_along_axis(tokens_mask, idx_cl, axis=1)   # [B, wl]
    # l-major gathered token windows, fp16: [wl, B, C]
    tokens_w = tokens[idx_cl.T, np.arange(B)[None, :], :].astype(np.float16)
    idx_c = ws[:, None] + np.arange(WL + 2 * PAD)
    idx_ccl = np.clip(idx_c, 0, T + 2 * PAD - 1)
    cum_w = np.take_along_axis(cum, idx_ccl, axis=1)           # [B, wl+2P]
    loc_in = (cum_w / np.float32(AVG) - np.float32(1.0)).astype(np.float16)
    madd = np.where(mask_w, np.float32(0.0), np.float32(-1e30)).astype(np.float32)
    cmid = cum_w[:, PAD:PAD + WL]                               # [B, wl]
    mdc = np.concatenate([madd, cmid], axis=1)                  # [B, 258]

    # wqx (shared): Wq chunks | bvecT row 0 | Ws | conv taps
    wqx = np.zeros((128, WQX_COLS), np.float16)
    wqx[:, 0:1024] = Wq.T.reshape(8, 128, H).transpose(1, 0, 2).reshape(128, 8 * H)
    wqx[0, 1024:1152] = (bq + conv_b).astype(np.float16)
    wqx[:, 1152] = Ws.astype(np.float16)
    wqx[0:K, 1153:1281] = conv_w[:, 0, :].T.astype(np.float16)

    in_maps = []
    for core in range(N_CORES):
        bs = slice(core * BL, (core + 1) * BL)
        q_core = query[0, bs, :]                               # [BL, QH]
        qt = np.ascontiguousarray(
            q_core.T.reshape(8, 128, BL).transpose(1, 0, 2).reshape(128, 8 * BL)
        ).astype(np.float16)
        # cimc: rows 0..8 im2col of this core's window, rows 32..47 selector
        cimc = np.zeros((CIMC_ROWS, BL, WL), np.float16)
        li = loc_in[bs]                                        # [BL, wl+2P]
        for k in range(K):
            cimc[k] = li[:, k:k + WL]
        for b in range(BL):
            cimc[32 + b, b, :] = 1.0
        in_maps.append({
            "tok": np.ascontiguousarray(tokens_w[:, bs, :]),
            "cimc": cimc.reshape(CIMC_ROWS, NBL),
            "qt": qt,
            "wqx": wqx,
            "mdc": np.ascontiguousarray(mdc[bs]),
        })
    host_ctx = {
        "idx": idx, "cum": cum, "ws_dtype": np.asarray(inputs["window_start"]).dtype,
        "window_start": np.asarray(inputs["window_start"]),
        "num_tokens": np.asarray(inputs["num_tokens"]),
    }
    return in_maps, host_ctx


def _assemble(results, host_ctx):
    ctx_out = np.concatenate([r["ctx_o"] for r in results], axis=0)   # [B, C]
    anw = np.concatenate([r["anw_o"] for r in results], axis=0)       # [B, 258]
    aln = anw[:, 0:WL]
    ncw = anw[:, WL:2 * WL]

    idx = host_ctx["idx"]
    cum = host_ctx["cum"]
    full_len = T + 2 * PAD
    pos = idx + PAD                                                    # [B, wl]
    full = np.zeros((B, full_len), np.float32)
    new_cum = cum.copy()
    inb = pos < full_len  # jax scatter drops OOB
    if inb.all():
        bidx = np.arange(B)[:, None]
        full[bidx, pos] = aln
        new_cum[bidx, pos] = ncw
    else:
        for b in range(B):
            p = pos[b][inb[b]]
            full[b, p] = aln[b][inb[b]]
            new_cum[b, p] = ncw[b][inb[b]]

    out_aln = full[:, PAD:-PAD]                                        # [B, T]
    ws_in = host_ctx["window_start"]
    num_tokens = host_ctx["num_tokens"]
    amax = np.argmax(full, axis=1)
    new_ws = amax.astype(np.int64) - WL // 2 - PAD
    new_ws = np.minimum(new_ws, np.asarray(num_tokens, np.int64) - WL)
    new_ws = np.clip(new_ws, 0, None)
    new_ws = np.maximum(np.asarray(ws_in, np.int64), new_ws)
    new_ws = new_ws.astype(host_ctx["ws_dtype"])
    return ctx_out, out_aln, new_cum, new_ws


def kernel(**inputs):
    nc = _get_nc(loop_n=1)
    in_maps, host_ctx = _prepare(inputs)
    res = run_bass_kernel_spmd(nc, in_maps, core_ids=list(range(N_CORES)))
    return _assemble(res.results, host_ctx)
